# revision 1
# baseline (speedup 1.0000x reference)
"""GCN-with-edge-MLP kernel for trn2, 8-core SPMD (self-contained).

Per core (convs dst-sharded; MLP sharded by original edge order):
  A': xs = bf16(dinv * x)                          (sequential pass)
  conv1: per node tile: indirect-gather xs[src] rows (incl. self-loop slots),
         one-hot (is_equal) + PE matmul aggregation in PSUM, epilogue -> q (2-wide)
  AllGather(q); conv2 same structure 2-wide -> p; AllGather(p)
  MLP: gather p[s], p[d]; embedding/vl/fcb1 part via host one-hot matmuls;
       relu -> 2x2 fc -> log_softmax, stored in original edge order.

Algebraic collapses (validated against the reference numerically):
  h1 = relu(dinv*(agg(xs) @ W1) + b1),  xs = dinv*x
  q  = dinv*(h1 @ (W2 @ fcW1[:64]));   p = dinv*agg(q)   (b2@Wn cancels in p[s]-p[d])
  logits1 = p[s]-p[d] + Te0[a2]+Te1[a3] + a0*fcW1[64]+a1*fcW1[65] + fcb1
  out = log_softmax(relu(logits1) @ fcW2 + fcb2)
"""
import numpy as np
import ml_dtypes

import concourse.bacc as bacc
import concourse.bass as bass
import concourse.mybir as mybir
import concourse.tile as tile
from concourse.bass_utils import run_bass_kernel_spmd
from contextlib import ExitStack

dt = mybir.dt
bf16 = ml_dtypes.bfloat16
NCORES = 8
AF = mybir.ActivationFunctionType
ALU = mybir.AluOpType


def _host_prep(x, edge_index, edge_attr, C=17, CM=157):
    N = x.shape[0]
    E = edge_index.shape[1]
    src = np.asarray(edge_index[0], dtype=np.int64)
    dst = np.asarray(edge_index[1], dtype=np.int64)
    ea = np.asarray(edge_attr, dtype=np.int64)
    deg = np.bincount(dst, minlength=N).astype(np.float32) + 1.0

    cnt = np.bincount(dst, minlength=N) + 1          # +1 self slot per node
    cume = np.cumsum(cnt)
    bounds = [0]
    for k in range(1, NCORES):
        bounds.append(int(np.searchsorted(cume, cume[-1] * k / NCORES)))
    bounds.append(N)

    budget = C * 128
    tiles = []
    for k in range(NCORES):
        lo, hi = bounds[k], bounds[k + 1]
        t = []
        n0 = lo
        while n0 < hi:
            n1 = min(n0 + 128, hi)
            while cnt[n0:n1].sum() > budget and n1 > n0 + 1:
                n1 -= 1
            t.append((n0, n1))
            n0 = n1
        tiles.append(t)
    T = max(len(t) for t in tiles)
    for t in tiles:
        while len(t) < T:
            t.append((N, N))

    rowof = np.zeros(N, dtype=np.int64)
    deg_pk = np.ones((NCORES, 128, T), dtype=np.float32)
    for k in range(NCORES):
        for ti, (n0, n1) in enumerate(tiles[k]):
            if n0 < N:
                idx = np.arange(n0, n1)
                rowof[idx] = k * T * 128 + ti * 128 + (idx - n0)
                deg_pk[k, :n1 - n0, ti] = deg[n0:n1]

    order = np.argsort(dst, kind="stable")
    s_sorted, d_sorted = src[order], dst[order]
    srcx = np.zeros((NCORES, T, 128, C), dtype=np.int32)
    srcq = np.zeros((NCORES, T, 128, C), dtype=np.int32)
    dstloc = np.full((NCORES, T, 128, C), 300.0, dtype=np.float32)
    ptr = np.searchsorted(d_sorted, np.arange(N + 1))
    for k in range(NCORES):
        for ti, (n0, n1) in enumerate(tiles[k]):
            if n0 >= N:
                continue
            a, b = ptr[n0], ptr[n1]
            # slot list = real edges + self slots
            ss = np.concatenate([s_sorted[a:b], np.arange(n0, n1)])
            dd = np.concatenate([d_sorted[a:b], np.arange(n0, n1)])
            L = len(ss)
            kk = np.arange(L)
            pp, cc = kk % 128, kk // 128
            srcx[k, ti, pp, cc] = ss
            srcq[k, ti, pp, cc] = rowof[ss]
            dstloc[k, ti, pp, cc] = (dd - n0).astype(np.float32)

    EPC = E // NCORES
    TM = int(np.ceil(EPC / (128 * CM)))
    sprow = np.zeros((NCORES, TM, 128, CM), dtype=np.int32)
    dprow = np.zeros((NCORES, TM, 128, CM), dtype=np.int32)
    attr = np.zeros((NCORES, TM, 48, CM * 128), dtype=bf16)
    a0, a1, a2, a3 = ea[0], ea[1], ea[2], ea[3]
    for k in range(NCORES):
        ids = np.arange(EPC)
        tm = ids // (128 * CM)
        rem = ids % (128 * CM)
        pp = rem // CM
        cc = rem % CM
        ge = k * EPC + ids
        sprow[k, tm, pp, cc] = rowof[src[ge]]
        dprow[k, tm, pp, cc] = rowof[dst[ge]]
        sl = cc * 128 + pp
        attr[k, tm, a2[ge], sl] = 1.0
        attr[k, tm, 20 + a3[ge], sl] = 1.0
        attr[k, tm, 40, sl] = a0[ge].astype(np.float32).astype(bf16)
        attr[k, tm, 41, sl] = a1[ge].astype(np.float32).astype(bf16)
        attr[k, tm, 42, sl] = 1.0

    NTA = int(np.ceil(N / 2048))
    XP = NTA * 2048
    NT0 = XP // 128
    degT = np.ones((128, NT0), dtype=np.float32)
    nn = np.arange(N)
    degT[nn % 128, nn // 128] = deg
    xpad = np.zeros((XP, x.shape[1]), dtype=np.float32)
    xpad[:N] = np.asarray(x, dtype=np.float32)

    iota128 = np.broadcast_to(np.arange(128, dtype=np.float32), (128, 128)).astype(bf16).copy()
    ident128 = np.eye(128, dtype=np.float32).astype(bf16)

    meta = dict(T=T, C=C, TM=TM, CM=CM, NTA=NTA, XP=XP, NT0=NT0, EPC=EPC)
    percore = []
    for k in range(NCORES):
        percore.append(dict(
            x=xpad, degT=degT,
            deg_pk=np.ascontiguousarray(deg_pk[k]),
            srcx=np.ascontiguousarray(srcx[k]),
            srcq=np.ascontiguousarray(srcq[k]),
            dstloc=np.ascontiguousarray(dstloc[k].astype(bf16)),
            sprow=np.ascontiguousarray(sprow[k]),
            dprow=np.ascontiguousarray(dprow[k]),
            attr=np.ascontiguousarray(attr[k]),
            iota128=iota128, ident128=ident128,
        ))
    return percore, meta


def _build(meta, fcW2, fcb2, F=64):
    T, C, TM, CM = meta["T"], meta["C"], meta["TM"], meta["CM"]
    NTA, XP, NT0 = meta["NTA"], meta["XP"], meta["NT0"]
    TG = NCORES * T * 128

    nc = bacc.Bacc("TRN2", target_bir_lowering=False, debug=False, num_devices=NCORES)

    t_x = nc.dram_tensor("x", [XP, F], dt.float32, kind="ExternalInput")
    t_degT = nc.dram_tensor("degT", [128, NT0], dt.float32, kind="ExternalInput")
    t_degpk = nc.dram_tensor("deg_pk", [128, T], dt.float32, kind="ExternalInput")
    t_srcx = nc.dram_tensor("srcx", [T, 128, C], dt.int32, kind="ExternalInput")
    t_srcq = nc.dram_tensor("srcq", [T, 128, C], dt.int32, kind="ExternalInput")
    t_dstloc = nc.dram_tensor("dstloc", [T, 128, C], dt.bfloat16, kind="ExternalInput")
    t_sprow = nc.dram_tensor("sprow", [TM, 128, CM], dt.int32, kind="ExternalInput")
    t_dprow = nc.dram_tensor("dprow", [TM, 128, CM], dt.int32, kind="ExternalInput")
    t_attr = nc.dram_tensor("attr", [TM, 48, CM * 128], dt.bfloat16, kind="ExternalInput")
    t_iota = nc.dram_tensor("iota128", [128, 128], dt.bfloat16, kind="ExternalInput")
    t_ident = nc.dram_tensor("ident128", [128, 128], dt.bfloat16, kind="ExternalInput")
    t_W1b = nc.dram_tensor("W1b", [F, F], dt.bfloat16, kind="ExternalInput")
    t_W2nb = nc.dram_tensor("W2nb", [F, 2], dt.bfloat16, kind="ExternalInput")
    t_b1rep = nc.dram_tensor("b1rep", [128, F], dt.float32, kind="ExternalInput")
    t_TeS = nc.dram_tensor("TeS", [48, 2], dt.bfloat16, kind="ExternalInput")

    t_out = nc.dram_tensor("out", [TM * 128 * CM, 2], dt.float32, kind="ExternalOutput")

    d_xs = nc.dram_tensor("xs_i", [XP, F], dt.bfloat16)
    d_qloc = nc.dram_tensor("qloc_i", [T * 128, 2], dt.bfloat16)
    d_qfull = nc.dram_tensor("qfull_i", [TG, 2], dt.bfloat16, addr_space="Shared")
    d_ploc = nc.dram_tensor("ploc_i", [T * 128, 2], dt.float32)
    d_pfull = nc.dram_tensor("pfull_i", [TG, 2], dt.float32, addr_space="Shared")

    w00, w01 = float(fcW2[0, 0]), float(fcW2[0, 1])
    w10, w11 = float(fcW2[1, 0]), float(fcW2[1, 1])
    bb0, bb1 = float(fcb2[0]), float(fcb2[1])

    with tile.TileContext(nc) as tc, ExitStack() as ctx:
        cst = ctx.enter_context(tc.tile_pool(name="cst", bufs=1))
        wk = ctx.enter_context(tc.tile_pool(name="wk", bufs=4))
        gp = ctx.enter_context(tc.tile_pool(name="gp", bufs=3))
        mg = ctx.enter_context(tc.tile_pool(name="mg", bufs=2))
        psA = ctx.enter_context(tc.tile_pool(name="psA", bufs=1, space="PSUM"))
        psB = ctx.enter_context(tc.tile_pool(name="psB", bufs=2, space="PSUM"))

        iota_t = cst.tile([128, 128], dt.bfloat16, tag="iota")
        nc.sync.dma_start(iota_t[:], t_iota[:, :])
        ident_t = cst.tile([128, 128], dt.bfloat16, tag="ident")
        nc.sync.dma_start(ident_t[:], t_ident[:, :])
        W1b_t = cst.tile([F, F], dt.bfloat16, tag="W1b")
        nc.sync.dma_start(W1b_t[:], t_W1b[:, :])
        W2nb_t = cst.tile([F, 2], dt.bfloat16, tag="W2nb")
        nc.sync.dma_start(W2nb_t[:], t_W2nb[:, :])
        b1rep_t = cst.tile([128, F], dt.float32, tag="b1rep")
        nc.sync.dma_start(b1rep_t[:], t_b1rep[:, :])
        TeS_t = cst.tile([48, 2], dt.bfloat16, tag="TeS")
        nc.sync.dma_start(TeS_t[:], t_TeS[:, :])

        degT_t = cst.tile([128, NT0], dt.float32, tag="degT")
        nc.sync.dma_start(degT_t[:], t_degT[:, :])
        rec_t = cst.tile([128, NT0], dt.float32, tag="recT")
        nc.vector.reciprocal(rec_t[:], degT_t[:])
        dinvO_t = cst.tile([128, NT0], dt.float32, tag="dinvO")
        nc.scalar.activation(dinvO_t[:], rec_t[:], AF.Sqrt)

        degpk_t = cst.tile([128, T], dt.float32, tag="degpk")
        nc.sync.dma_start(degpk_t[:], t_degpk[:, :])
        recp_t = cst.tile([128, T], dt.float32, tag="recp")
        nc.vector.reciprocal(recp_t[:], degpk_t[:])
        dinvP_t = cst.tile([128, T], dt.float32, tag="dinvP")
        nc.scalar.activation(dinvP_t[:], recp_t[:], AF.Sqrt)

        # preload conv index/dstloc arrays (no dependence on xs)
        sx_all = cst.tile([128, T * C], dt.int32, tag="sxall")
        nc.sync.dma_start(sx_all[:].rearrange("p (t c) -> p t c", c=C), t_srcx[:, :, :].rearrange("t p c -> p t c"))
        sq_all = cst.tile([128, T * C], dt.int32, tag="sqall")
        nc.sync.dma_start(sq_all[:].rearrange("p (t c) -> p t c", c=C), t_srcq[:, :, :].rearrange("t p c -> p t c"))
        dl_all = cst.tile([128, T * C], dt.bfloat16, tag="dlall")
        nc.sync.dma_start(dl_all[:].rearrange("p (t c) -> p t c", c=C), t_dstloc[:, :, :].rearrange("t p c -> p t c"))

        # ---- phase A' ----
        for i in range(NTA):
            xt = wk.tile([128, 16 * F], dt.float32, tag="ax")
            nc.sync.dma_start(
                xt[:].rearrange("p (s f) -> p s f", f=F),
                t_x[:, :].rearrange("(i s p) f -> i p s f", p=128, s=16)[i])
            xo = wk.tile([128, 16 * F], dt.bfloat16, tag="axo")
            nc.vector.tensor_tensor(
                out=xo[:].rearrange("p (s f) -> p s f", f=F),
                in0=xt[:].rearrange("p (s f) -> p s f", f=F),
                in1=dinvO_t[:, i * 16:(i + 1) * 16].broadcast_to((128, 16, F)),
                op=ALU.mult)
            nc.sync.dma_start(
                d_xs[:XP, :].rearrange("(i s p) f -> i p s f", p=128, s=16)[i], xo[:])
        tc.strict_bb_all_engine_barrier()

        def conv_pass(src_all, table_dram, width, WG, out_cb):
            for t in range(T):
                gts = []
                for c in range(C):
                    g_c = gp.tile([128, WG], dt.bfloat16, tag=f"cg{c % 4}")
                    nc.gpsimd.indirect_dma_start(
                        out=g_c[:], out_offset=None,
                        in_=table_dram[:, :],
                        in_offset=bass.IndirectOffsetOnAxis(
                            ap=src_all[:, t * C + c:t * C + c + 1], axis=0))
                    gts.append(g_c)
                dl_t = dl_all[:, t * C:(t + 1) * C]
                oh_t = gp.tile([128, C * 128], dt.bfloat16, tag="coh")
                nc.vector.tensor_tensor(
                    out=oh_t[:].rearrange("p (c j) -> p c j", j=128),
                    in0=dl_t.broadcast_to((128, C, 128)),
                    in1=iota_t[:].rearrange("p (o j) -> p o j", o=1).broadcast_to((128, C, 128)),
                    op=ALU.is_equal)
                aggT = psA.tile([width, 128], dt.float32, tag="aggT")
                for c in range(C):
                    nc.tensor.matmul(
                        out=aggT[:],
                        lhsT=gts[c][:, 0:width],
                        rhs=oh_t[:, c * 128:(c + 1) * 128],
                        start=(c == 0), stop=(c == C - 1))
                out_cb(t, aggT)

        def conv1_out(t, aggT):
            aggb = wk.tile([F, 128], dt.bfloat16, tag="aggb")
            nc.vector.tensor_copy(aggb[:], aggT[:])
            h1p = psA.tile([128, F], dt.float32, tag="h1p")
            nc.tensor.matmul(out=h1p[:], lhsT=aggb[:], rhs=W1b_t[:], start=True, stop=True)
            h1s = wk.tile([128, F], dt.float32, tag="h1s")
            nc.vector.tensor_scalar(out=h1s[:], in0=h1p[:],
                                    scalar1=dinvP_t[:, t:t + 1], scalar2=None,
                                    op0=ALU.mult)
            h1a = wk.tile([128, F], dt.float32, tag="h1a")
            nc.vector.tensor_tensor(out=h1a[:], in0=h1s[:], in1=b1rep_t[:], op=ALU.add)
            h1b = wk.tile([128, F], dt.bfloat16, tag="h1b")
            nc.scalar.activation(h1b[:], h1a[:], AF.Relu)
            h1Tp = psA.tile([F, 128], dt.bfloat16, tag="h1Tp")
            nc.tensor.transpose(out=h1Tp[:], in_=h1b[:], identity=ident_t[:])
            h1Tb = wk.tile([F, 128], dt.bfloat16, tag="h1Tb")
            nc.vector.tensor_copy(h1Tb[:], h1Tp[:])
            qp = psA.tile([128, 2], dt.float32, tag="qp")
            nc.tensor.matmul(out=qp[:], lhsT=h1Tb[:], rhs=W2nb_t[:], start=True, stop=True)
            qsb = wk.tile([128, 2], dt.bfloat16, tag="qsb")
            nc.vector.tensor_scalar(out=qsb[:], in0=qp[:],
                                    scalar1=dinvP_t[:, t:t + 1], scalar2=None,
                                    op0=ALU.mult)
            nc.sync.dma_start(d_qloc[t * 128:(t + 1) * 128, 0:2], qsb[:])

        conv_pass(sx_all, d_xs, F, F, conv1_out)
        tc.strict_bb_all_engine_barrier()

        nc.gpsimd.collective_compute(
            "AllGather", ALU.bypass, replica_groups=[list(range(NCORES))],
            ins=[d_qloc[:, :].opt()], outs=[d_qfull[:, :].opt()])
        tc.strict_bb_all_engine_barrier()

        def conv2_out(t, aggT):
            a2b = wk.tile([2, 128], dt.bfloat16, tag="a2b")
            nc.vector.tensor_copy(a2b[:], aggT[:])
            pT = psB.tile([128, 2], dt.bfloat16, tag="pT")
            nc.tensor.transpose(out=pT[:], in_=a2b[:], identity=ident_t[0:2, 0:2])
            psb = wk.tile([128, 2], dt.float32, tag="psb")
            nc.vector.tensor_scalar(out=psb[:], in0=pT[:],
                                    scalar1=dinvP_t[:, t:t + 1], scalar2=None,
                                    op0=ALU.mult)
            nc.sync.dma_start(d_ploc[t * 128:(t + 1) * 128, 0:2], psb[:])

        conv_pass(sq_all, d_qfull, 2, 2, conv2_out)
        tc.strict_bb_all_engine_barrier()

        nc.gpsimd.collective_compute(
            "AllGather", ALU.bypass, replica_groups=[list(range(NCORES))],
            ins=[d_ploc[:, :].opt()], outs=[d_pfull[:, :].opt()])
        tc.strict_bb_all_engine_barrier()

        # ---- MLP ----
        for tm in range(TM):
            spi = wk.tile([128, CM], dt.int32, tag="spi")
            nc.sync.dma_start(spi[:], t_sprow[tm])
            dpi = wk.tile([128, CM], dt.int32, tag="dpi")
            nc.sync.dma_start(dpi[:], t_dprow[tm])
            at_t = mg.tile([48, CM * 128], dt.bfloat16, tag="attr")
            nc.sync.dma_start(at_t[:], t_attr[tm])
            psg = mg.tile([128, CM * 2], dt.float32, tag="psg")
            pdg = mg.tile([128, CM * 2], dt.float32, tag="pdg")
            for c in range(CM):
                nc.gpsimd.indirect_dma_start(
                    out=psg[:, c * 2:(c + 1) * 2], out_offset=None,
                    in_=d_pfull[:, :],
                    in_offset=bass.IndirectOffsetOnAxis(ap=spi[:, c:c + 1], axis=0))
                nc.gpsimd.indirect_dma_start(
                    out=pdg[:, c * 2:(c + 1) * 2], out_offset=None,
                    in_=d_pfull[:, :],
                    in_offset=bass.IndirectOffsetOnAxis(ap=dpi[:, c:c + 1], axis=0))
            te_sb = mg.tile([128, CM * 2], dt.float32, tag="te")
            for c0 in range(0, CM, 8):
                tep = psB.tile([128, 16], dt.float32, tag="tep")
                for c in range(c0, min(c0 + 8, CM)):
                    nc.tensor.matmul(
                        out=tep[:, (c - c0) * 2:(c - c0) * 2 + 2],
                        lhsT=at_t[:, c * 128:(c + 1) * 128],
                        rhs=TeS_t[:], start=True, stop=True)
                wte = min(16, (CM - c0) * 2)
                nc.vector.tensor_copy(te_sb[:, c0 * 2:c0 * 2 + wte], tep[:, :wte])
            dl = mg.tile([128, CM * 2], dt.float32, tag="dl")
            nc.vector.tensor_tensor(out=dl[:], in0=psg[:], in1=pdg[:], op=ALU.subtract)
            l1 = mg.tile([128, CM * 2], dt.float32, tag="l1")
            nc.vector.tensor_tensor(out=l1[:], in0=dl[:], in1=te_sb[:], op=ALU.add)
            r = mg.tile([128, CM * 2], dt.float32, tag="r")
            nc.scalar.activation(r[:], l1[:], AF.Relu)
            r3 = r[:].rearrange("p (c two) -> p c two", two=2)
            r0, r1 = r3[:, :, 0:1], r3[:, :, 1:2]

            def col(tag):
                tt = mg.tile([128, CM], dt.float32, tag=tag)
                return tt, tt[:].rearrange("p (c o) -> p c o", o=1)

            o0, o0v = col("o0")
            nc.vector.tensor_scalar(out=o0v, in0=r0, scalar1=w00, scalar2=bb0,
                                    op0=ALU.mult, op1=ALU.add)
            tmp0, tmp0v = col("tmp0")
            nc.vector.tensor_scalar(out=tmp0v, in0=r1, scalar1=w10, scalar2=None,
                                    op0=ALU.mult)
            o0b = mg.tile([128, CM], dt.float32, tag="o0b")
            nc.vector.tensor_tensor(out=o0b[:], in0=o0[:], in1=tmp0[:], op=ALU.add)
            o1, o1v = col("o1")
            nc.vector.tensor_scalar(out=o1v, in0=r0, scalar1=w01, scalar2=bb1,
                                    op0=ALU.mult, op1=ALU.add)
            tmp1, tmp1v = col("tmp1")
            nc.vector.tensor_scalar(out=tmp1v, in0=r1, scalar1=w11, scalar2=None,
                                    op0=ALU.mult)
            o1b = mg.tile([128, CM], dt.float32, tag="o1b")
            nc.vector.tensor_tensor(out=o1b[:], in0=o1[:], in1=tmp1[:], op=ALU.add)
            mx = mg.tile([128, CM], dt.float32, tag="mx")
            nc.vector.tensor_tensor(out=mx[:], in0=o0b[:], in1=o1b[:], op=ALU.max)
            s0 = mg.tile([128, CM], dt.float32, tag="s0")
            nc.vector.tensor_tensor(out=s0[:], in0=o0b[:], in1=mx[:], op=ALU.subtract)
            s1 = mg.tile([128, CM], dt.float32, tag="s1")
            nc.vector.tensor_tensor(out=s1[:], in0=o1b[:], in1=mx[:], op=ALU.subtract)
            e0t = mg.tile([128, CM], dt.float32, tag="e0t")
            nc.scalar.activation(e0t[:], s0[:], AF.Exp)
            e1t = mg.tile([128, CM], dt.float32, tag="e1t")
            nc.scalar.activation(e1t[:], s1[:], AF.Exp)
            se = mg.tile([128, CM], dt.float32, tag="se")
            nc.vector.tensor_tensor(out=se[:], in0=e0t[:], in1=e1t[:], op=ALU.add)
            ls = mg.tile([128, CM], dt.float32, tag="ls")
            nc.scalar.activation(ls[:], se[:], AF.Ln)
            ot = mg.tile([128, CM * 2], dt.float32, tag="ot")
            ot3 = ot[:].rearrange("p (c two) -> p c two", two=2)
            lsv = ls[:].rearrange("p (c o) -> p c o", o=1)
            nc.vector.tensor_tensor(out=ot3[:, :, 0:1],
                                    in0=s0[:].rearrange("p (c o) -> p c o", o=1),
                                    in1=lsv, op=ALU.subtract)
            nc.vector.tensor_tensor(out=ot3[:, :, 1:2],
                                    in0=s1[:].rearrange("p (c o) -> p c o", o=1),
                                    in1=lsv, op=ALU.subtract)
            nc.sync.dma_start(
                t_out[:, :].rearrange("(t p c) two -> t p (c two)", p=128, c=CM)[tm],
                ot[:])

    nc.compile()
    return nc


def kernel(x, edge_index, edge_attr, W1, b1, W2, b2, emb0, emb1,
           fcW1, fcb1, fcW2, fcb2, _sim=False, _prep_only=False):
    x = np.asarray(x, dtype=np.float32)
    W1 = np.asarray(W1, dtype=np.float32)
    b1 = np.asarray(b1, dtype=np.float32)
    W2 = np.asarray(W2, dtype=np.float32)
    fcW1 = np.asarray(fcW1, dtype=np.float32)
    fcb1 = np.asarray(fcb1, dtype=np.float32)
    fcW2 = np.asarray(fcW2, dtype=np.float32)
    fcb2 = np.asarray(fcb2, dtype=np.float32)
    emb0 = np.asarray(emb0, dtype=np.float32)
    emb1 = np.asarray(emb1, dtype=np.float32)

    small = x.shape[0] <= 4096
    percore, meta = _host_prep(x, np.asarray(edge_index), np.asarray(edge_attr),
                               C=4 if small else 17, CM=8 if small else 157)

    Wn = fcW1[:64]
    W2n = (W2 @ Wn).astype(np.float32)
    TeS = np.zeros((48, 2), dtype=np.float32)
    TeS[0:20] = emb0 @ fcW1[66:98]
    TeS[20:40] = emb1 @ fcW1[98:130]
    TeS[40] = fcW1[64]
    TeS[41] = fcW1[65]
    TeS[42] = fcb1
    for m in percore:
        m["W1b"] = W1.astype(bf16)
        m["W2nb"] = W2n.astype(bf16)
        m["b1rep"] = np.broadcast_to(b1, (128, 64)).astype(np.float32).copy()
        m["TeS"] = TeS.astype(bf16)

    nc = _build(meta, fcW2, fcb2)
    if _prep_only:
        return nc, percore, meta

    if _sim:
        from concourse.bass_interp import MultiCoreSim
        sim = MultiCoreSim(nc, NCORES)
        for k in range(NCORES):
            for name, v in percore[k].items():
                sim.cores[k].tensor(name)[:] = v
        sim.simulate(check_with_hw=False)
        outs = [np.asarray(sim.cores[k].mem_tensor("out")) for k in range(NCORES)]
    else:
        res = run_bass_kernel_spmd(nc, percore, core_ids=list(range(NCORES)))
        outs = [np.asarray(res.results[k]["out"]) for k in range(NCORES)]
    EPC = meta["EPC"]
    return np.concatenate([o[:EPC] for o in outs], axis=0).astype(np.float32)



# revision 13
# speedup vs baseline: 2.1987x; 2.1987x over previous
"""GCN-with-edge-MLP kernel for trn2, 8-core SPMD (self-contained).

Equal node shards (12544 nodes/core, row = node id everywhere). Per core:
  conv1: per 128-node dst tile, slots (edges+self, grouped by src quarter
         window so indices fit int16) gathered from x fp32 rows (256B) via
         4-queue dma_gather, scaled by host norm (dinv_s*dinv_d) on the Pool
         engine, aggregated with one-hot (is_equal) matmuls on PE ->
         h1 = relu(agg@W1 + b1), q = h1 @ (W2@fcW1[:64]).
  AllGather(q); expand q into 256B-strided quarter tables (strided DMA).
  conv2: same slot structure and the SAME int16 indices, gather q rows,
         2-wide aggregation -> p. Fused pd expansion: PE-transpose each
         one-hot chunk, pd_c = ohT_c^T @ p_tile, kept in SBUF.
  AllGather(p); expand p likewise.
  MLP in slot order: gather p rows (same indices) -> ps; l = ps - pd + TeV
         (TeV = host-folded edge-attr term, like the baseline's TeS fold);
         2-class log_softmax via softplus: out = (-sp(-d), -sp(d)).
  Host drops self/pad slots and permutes the slot-ordered output back to
  original edge order (index bookkeeping only).

Algebraic collapses (validated against the reference numerically):
  h1 = relu(agg(norm*x) @ W1 + b1),  q = h1 @ (W2 @ fcW1[:64])  (b2 cancels)
  logits1 = p[s]-p[d] + TeV[e]; out = log_softmax(relu(logits1)@fcW2 + fcb2)
"""
import numpy as np
import ml_dtypes

import concourse.bacc as bacc
import concourse.bass as bass
import concourse.mybir as mybir
import concourse.tile as tile
from concourse.bass_utils import run_bass_kernel_spmd
from contextlib import ExitStack

dt = mybir.dt
bf16 = ml_dtypes.bfloat16
NCORES = 8
AF = mybir.ActivationFunctionType
ALU = mybir.AluOpType

N_NODES = 100000
XP = 100352          # padded nodes = 8 * 12544 = 4 * 25088
NSH = XP // NCORES   # 12544 nodes per core
QW = XP // 4         # 25088-row quarter windows (int16-safe)
TPC = NSH // 128     # 98 tiles per core
F = 64
MAXC = 32            # SBUF capacity in 128-slot chunks per tile


def _host_prep(x, edge_index, edge_attr, fcW1, fcb1, emb0, emb1):
    N = x.shape[0]
    src = np.asarray(edge_index[0], dtype=np.int64)
    dst = np.asarray(edge_index[1], dtype=np.int64)
    ea = np.asarray(edge_attr, dtype=np.int64)
    E = src.shape[0]

    deg = np.bincount(dst, minlength=XP).astype(np.float32) + 1.0
    dinv = 1.0 / np.sqrt(deg)

    fcW1 = np.asarray(fcW1, dtype=np.float32)
    Te0 = (np.asarray(emb0, dtype=np.float32) @ fcW1[66:98])
    Te1 = (np.asarray(emb1, dtype=np.float32) @ fcW1[98:130])
    TeV = (Te0[ea[2]] + Te1[ea[3]]
           + ea[0][:, None].astype(np.float32) * fcW1[64][None, :]
           + ea[1][:, None].astype(np.float32) * fcW1[65][None, :]
           + np.asarray(fcb1, dtype=np.float32)[None, :]).astype(np.float32)

    order = np.argsort(dst, kind="stable")
    s_sorted, d_sorted = src[order], dst[order]
    e_sorted = order
    ptr = np.searchsorted(d_sorted, np.arange(XP + 1))

    xpad = np.zeros((XP, F), dtype=np.float32)
    xpad[:N] = np.asarray(x, dtype=np.float32)

    # pass 1: per (core, tile, quarter) slot lists and counts
    lists = [[[None] * 4 for _ in range(TPC)] for _ in range(NCORES)]
    CQ = np.zeros((TPC, 4), dtype=np.int64)   # common chunk capacity
    for k in range(NCORES):
        lo = k * NSH
        for t in range(TPC):
            n0 = lo + t * 128
            n1 = n0 + 128
            a, b = ptr[n0], ptr[n1]
            ss = np.concatenate([s_sorted[a:b], np.arange(n0, n1)])
            dd = np.concatenate([d_sorted[a:b], np.arange(n0, n1)])
            ee = np.concatenate([e_sorted[a:b], np.full(128, -1, np.int64)])
            qt = ss // QW
            for q in range(4):
                m = qt == q
                lists[k][t][q] = (ss[m], dd[m], ee[m])
                CQ[t, q] = max(CQ[t, q], (int(m.sum()) + 127) // 128)
    assert CQ.sum(axis=1).max() <= 30

    # common structure
    instr_meta = []   # (t, q, Cq, off16)
    off16 = 0
    tile_C = CQ.sum(axis=1).astype(np.int64)
    for t in range(TPC):
        for q in range(4):
            if CQ[t, q] == 0:
                continue
            instr_meta.append((t, q, int(CQ[t, q]), off16))
            off16 += int(CQ[t, q]) * 8   # (Cq*128)/16 int16 cols
    NIDX16 = off16
    CSUM = int(tile_C.sum())
    coff = np.concatenate([[0], np.cumsum(tile_C)]).astype(np.int64)

    # pass 2: per-core arrays at common capacities
    percore = []
    slotmaps = []
    for k in range(NCORES):
        idx_all = np.zeros((128, NIDX16), dtype=np.int16)
        dstloc = np.full((128, CSUM), 300.0, dtype=np.float32)
        normv = np.zeros((128, CSUM), dtype=np.float32)
        tev = np.zeros((128, CSUM, 2), dtype=np.float32)
        smap = np.full((128, CSUM), -1, dtype=np.int64)
        for (t, q, Cq, o16) in instr_meta:
            ss, dd, ee = lists[k][t][q]
            cnt = len(ss)
            cap = Cq * 128
            n0 = k * NSH + t * 128
            idx = np.zeros(cap, dtype=np.int16)
            idx[:cnt] = (ss - q * QW).astype(np.int16)
            w = idx.reshape(cap // 16, 16).T
            for g in range(8):
                idx_all[g * 16:(g + 1) * 16, o16:o16 + cap // 16] = w
            # column base for this quarter inside the tile
            cq0 = int(coff[t]) + int(CQ[t, :q].sum())
            kk = np.arange(cnt)
            pp, cc = kk % 128, cq0 + kk // 128
            dstloc[pp, cc] = (dd - n0).astype(np.float32)
            normv[pp, cc] = dinv[ss] * dinv[dd]
            real = ee >= 0
            tev[pp[real], cc[real]] = TeV[ee[real]]
            smap[pp, cc] = ee
        percore.append(dict(
            x=xpad, idx16=idx_all,
            dstloc=np.ascontiguousarray(dstloc.astype(bf16)),
            normv=np.ascontiguousarray(normv),
            tev=np.ascontiguousarray(tev.reshape(128, CSUM * 2))))
        slotmaps.append(smap)

    meta = dict(instr_meta=instr_meta, tile_C=tile_C, coff=coff,
                CSUM=CSUM, NIDX16=NIDX16, E=E)
    return percore, slotmaps, meta


def _build(meta, fcW2, fcb2):
    instr_meta = meta["instr_meta"]
    tile_C = meta["tile_C"]
    coff = meta["coff"]
    CSUM = meta["CSUM"]
    NIDX16 = meta["NIDX16"]

    wd0 = float(fcW2[0, 0] - fcW2[0, 1])
    wd1 = float(fcW2[1, 0] - fcW2[1, 1])
    bd = float(fcb2[0] - fcb2[1])

    nc = bacc.Bacc("TRN2", target_bir_lowering=False, debug=False,
                   num_devices=NCORES, num_swdge_queues=4)

    t_x = nc.dram_tensor("x", [XP, F], dt.float32, kind="ExternalInput")
    t_idx = nc.dram_tensor("idx16", [128, NIDX16], dt.int16, kind="ExternalInput")
    t_dl = nc.dram_tensor("dstloc", [128, CSUM], dt.bfloat16, kind="ExternalInput")
    t_nv = nc.dram_tensor("normv", [128, CSUM], dt.float32, kind="ExternalInput")
    t_tev = nc.dram_tensor("tev", [128, CSUM * 2], dt.float32, kind="ExternalInput")
    t_W1b = nc.dram_tensor("W1b", [F, F], dt.bfloat16, kind="ExternalInput")
    t_W2nb = nc.dram_tensor("W2nb", [F, 2], dt.bfloat16, kind="ExternalInput")
    t_b1c = nc.dram_tensor("b1c", [F, 1], dt.float32, kind="ExternalInput")
    t_iota = nc.dram_tensor("iota128", [128, 128], dt.bfloat16, kind="ExternalInput")
    t_ident = nc.dram_tensor("ident128", [128, 128], dt.bfloat16, kind="ExternalInput")
    t_wd = nc.dram_tensor("wd", [128, 6], dt.float32, kind="ExternalInput")
    t_idf = nc.dram_tensor("idf", [2, 2], dt.float32, kind="ExternalInput")

    t_out = nc.dram_tensor("out", [128, CSUM * 2], dt.float32, kind="ExternalOutput")


    d_qloc = nc.dram_tensor("qloc_i", [NSH, 2], dt.float32)
    d_qfull = nc.dram_tensor("qfull_i", [XP, 2], dt.float32, addr_space="Shared")
    d_ploc = nc.dram_tensor("ploc_i", [NSH, 2], dt.float32)
    d_pfull = nc.dram_tensor("pfull_i", [XP, 2], dt.float32, addr_space="Shared")
    d_qexp = [nc.dram_tensor(f"qexp{i}", [QW, F], dt.float32) for i in range(4)]
    d_pexp = [nc.dram_tensor(f"pexp{i}", [QW, F], dt.float32) for i in range(4)]

    tile_instrs = [[] for _ in range(TPC)]
    for (t, q, Cq, o16) in instr_meta:
        tile_instrs[t].append((q, Cq, o16))

    with tile.TileContext(nc) as tc, ExitStack() as ctx:
        cst = ctx.enter_context(tc.tile_pool(name="cst", bufs=1))
        wk = ctx.enter_context(tc.tile_pool(name="wk", bufs=2))
        gp = ctx.enter_context(tc.tile_pool(name="gp", bufs=2))
        ohp = ctx.enter_context(tc.tile_pool(name="ohp", bufs=2))
        psA = ctx.enter_context(tc.tile_pool(name="psA", bufs=3, space="PSUM"))
        psB = ctx.enter_context(tc.tile_pool(name="psB", bufs=1, space="PSUM"))
        psC = ctx.enter_context(tc.tile_pool(name="psC", bufs=2, space="PSUM"))
        psD = ctx.enter_context(tc.tile_pool(name="psD", bufs=2, space="PSUM"))

        iota_t = cst.tile([128, 128], dt.bfloat16, tag="iota")
        nc.sync.dma_start(iota_t[:], t_iota[:, :])
        ident_t = cst.tile([128, 128], dt.bfloat16, tag="ident")
        nc.sync.dma_start(ident_t[:], t_ident[:, :])
        idf_t = cst.tile([2, 2], dt.float32, tag="idf")
        nc.sync.dma_start(idf_t[:], t_idf[:, :])
        W1b_t = cst.tile([F, F], dt.bfloat16, tag="W1b")
        nc.sync.dma_start(W1b_t[:], t_W1b[:, :])
        W2nb_t = cst.tile([F, 2], dt.bfloat16, tag="W2nb")
        nc.sync.dma_start(W2nb_t[:], t_W2nb[:, :])
        b1c_t = cst.tile([F, 1], dt.float32, tag="b1c")
        nc.sync.dma_start(b1c_t[:], t_b1c[:, :])
        wd_t = cst.tile([128, 6], dt.float32, tag="wd")
        nc.sync.dma_start(wd_t[:], t_wd[:, :])

        idx_t = cst.tile([128, NIDX16], dt.int16, tag="idx")
        nc.sync.dma_start(idx_t[:], t_idx[:, :])
        dl_t = cst.tile([128, CSUM], dt.bfloat16, tag="dl")
        nc.sync.dma_start(dl_t[:], t_dl[:, :])
        nv_t = cst.tile([128, CSUM], dt.float32, tag="nv")
        nc.sync.dma_start(nv_t[:], t_nv[:, :])
        tev_t = cst.tile([128, CSUM * 2], dt.float32, tag="tev")
        nc.sync.dma_start(tev_t[:], t_tev[:, :])
        pdv_t = cst.tile([128, CSUM * 2], dt.float32, tag="pdv")

        def gathers(t, table_list):
            g = gp.tile([128, MAXC * F], dt.float32, tag="g")
            co = 0
            for gi, (q, Cq, o16) in enumerate(tile_instrs[t]):
                n = Cq * 128
                tab = table_list[q]
                nc.gpsimd.dma_gather(
                    out_ap=g[:, co * F:(co + Cq) * F].rearrange(
                        "p (c f) -> p c f", f=F),
                    in_ap=tab if isinstance(tab, bass.AP) else tab[:, :],
                    idxs_ap=idx_t[:, o16:o16 + n // 16],
                    num_idxs=n, num_idxs_reg=n, elem_size=F,
                    queue_num=(t * 4 + gi) % 4)
                co += Cq
            return g

        def build_oh(t, C_t, c0):
            oh = ohp.tile([128, MAXC * 128], dt.bfloat16, tag="oh")
            nc.vector.tensor_tensor(
                out=oh[:, :C_t * 128].rearrange("p (c j) -> p c j", j=128),
                in0=dl_t[:, c0:c0 + C_t].broadcast_to((128, C_t, 128)),
                in1=iota_t[:].rearrange("p (o j) -> p o j", o=1).broadcast_to(
                    (128, C_t, 128)),
                op=ALU.is_equal)
            return oh

        # ---------------- conv1 ----------------
        for t in range(TPC):
            C_t = int(tile_C[t])
            c0 = int(coff[t])
            g = gathers(t, [t_x[i * QW:(i + 1) * QW, :] for i in range(4)])
            gbf = wk.tile([128, MAXC * F], dt.bfloat16, tag="gbf")
            nc.vector.tensor_tensor(
                out=gbf[:, :C_t * F].rearrange("p (c f) -> p c f", f=F),
                in0=g[:, :C_t * F].rearrange("p (c f) -> p c f", f=F),
                in1=nv_t[:, c0:c0 + C_t].rearrange(
                    "p (c o) -> p c o", o=1).broadcast_to((128, C_t, F)),
                op=ALU.mult)
            oh = build_oh(t, C_t, c0)
            aggT = psA.tile([F, 128], dt.float32, tag="p64")
            for c in range(C_t):
                nc.tensor.matmul(
                    out=aggT[:], lhsT=gbf[:, c * F:(c + 1) * F],
                    rhs=oh[:, c * 128:(c + 1) * 128],
                    start=(c == 0), stop=(c == C_t - 1))
            aggS = wk.tile([F, 128], dt.bfloat16, tag="aggS")
            nc.vector.tensor_copy(aggS[:], aggT[:])
            h1p = psA.tile([F, 128], dt.float32, tag="p64")
            nc.tensor.matmul(out=h1p[:], lhsT=W1b_t[:], rhs=aggS[:],
                             start=True, stop=True)
            h1r = wk.tile([F, 128], dt.bfloat16, tag="h1r")
            nc.scalar.activation(h1r[:], h1p[:], AF.Relu, bias=b1c_t[:, 0:1])
            qTp = psB.tile([2, 128], dt.float32, tag="p2")
            nc.tensor.matmul(out=qTp[:], lhsT=W2nb_t[:], rhs=h1r[:],
                             start=True, stop=True)
            qTs = wk.tile([2, 128], dt.float32, tag="qTs")
            nc.vector.tensor_copy(qTs[:], qTp[:])
            qmix = psC.tile([128, 64], dt.float32, tag="pMix")
            nc.tensor.transpose(out=qmix[:, 62:64], in_=qTs[:],
                                identity=idf_t[0:2, 0:2])
            qsb = wk.tile([128, 2], dt.float32, tag="qsb")
            nc.vector.tensor_copy(qsb[:], qmix[:, 62:64])
            nc.sync.dma_start(d_qloc[t * 128:(t + 1) * 128, 0:2], qsb[:])
        tc.strict_bb_all_engine_barrier()

        nc.gpsimd.collective_compute(
            "AllGather", ALU.bypass, replica_groups=[list(range(NCORES))],
            ins=[d_qloc[:, :].opt()], outs=[d_qfull[:, :].opt()])
        tc.strict_bb_all_engine_barrier()
        for i in range(4):
            nc.sync.dma_start(d_qexp[i][:, 0:2], d_qfull[i * QW:(i + 1) * QW, :])
        tc.strict_bb_all_engine_barrier()

        # ---------------- conv2 + fused pd ----------------
        for t in range(TPC):
            C_t = int(tile_C[t])
            c0 = int(coff[t])
            g = gathers(t, d_qexp)
            g2n = wk.tile([128, MAXC * 2], dt.bfloat16, tag="g2n")
            nc.vector.tensor_tensor(
                out=g2n[:, :C_t * 2].rearrange("p (c w) -> p c w", w=2),
                in0=g[:, :C_t * F].rearrange("p (c f) -> p c f", f=F)[:, :, 0:2],
                in1=nv_t[:, c0:c0 + C_t].rearrange(
                    "p (c o) -> p c o", o=1).broadcast_to((128, C_t, 2)),
                op=ALU.mult)
            oh = build_oh(t, C_t, c0)
            aggT2 = psB.tile([2, 128], dt.float32, tag="p2")
            for c in range(C_t):
                nc.tensor.matmul(
                    out=aggT2[:], lhsT=g2n[:, c * 2:(c + 1) * 2],
                    rhs=oh[:, c * 128:(c + 1) * 128],
                    start=(c == 0), stop=(c == C_t - 1))
            a2s = wk.tile([2, 128], dt.float32, tag="a2s")
            nc.vector.tensor_copy(a2s[:], aggT2[:])
            pdp = psC.tile([128, 64], dt.float32, tag="pMix")
            nc.tensor.transpose(out=pdp[:, 62:64], in_=a2s[:],
                                identity=idf_t[0:2, 0:2])
            p_sb = wk.tile([128, 2], dt.float32, tag="psb")
            nc.vector.tensor_copy(p_sb[:], pdp[:, 62:64])
            nc.sync.dma_start(d_ploc[t * 128:(t + 1) * 128, 0:2], p_sb[:])
            p_bf = wk.tile([128, 2], dt.float32, tag="pbf")
            nc.vector.tensor_copy(p_bf[:], pdp[:, 62:64])
            for c in range(C_t):
                ohTp = psD.tile([128, 128], dt.bfloat16, tag="ohTp")
                nc.tensor.transpose(out=ohTp[:], in_=oh[:, c * 128:(c + 1) * 128],
                                    identity=ident_t[:])
                ohTs = ohp.tile([128, 128], dt.float32, tag="ohTs")
                nc.vector.tensor_copy(ohTs[:], ohTp[:])
                nc.tensor.matmul(out=pdp[:, c * 2:(c + 1) * 2], lhsT=ohTs[:],
                                 rhs=p_bf[:], start=True, stop=True)
            nc.vector.tensor_copy(pdv_t[:, c0 * 2:(c0 + C_t) * 2],
                                  pdp[:, :C_t * 2])
        tc.strict_bb_all_engine_barrier()

        nc.gpsimd.collective_compute(
            "AllGather", ALU.bypass, replica_groups=[list(range(NCORES))],
            ins=[d_ploc[:, :].opt()], outs=[d_pfull[:, :].opt()])
        tc.strict_bb_all_engine_barrier()
        for i in range(4):
            nc.sync.dma_start(d_pexp[i][:, 0:2], d_pfull[i * QW:(i + 1) * QW, :])
        tc.strict_bb_all_engine_barrier()

        # ---------------- MLP (slot order) ----------------
        for t in range(TPC):
            C_t = int(tile_C[t])
            c0 = int(coff[t])
            g = gathers(t, d_pexp)
            ps3 = g[:, :C_t * F].rearrange("p (c f) -> p c f", f=F)[:, :, 0:2]
            l_ = wk.tile([128, MAXC * 2], dt.float32, tag="l")
            l3 = l_[:, :C_t * 2].rearrange("p (c w) -> p c w", w=2)
            nc.vector.tensor_tensor(
                out=l3, in0=ps3,
                in1=pdv_t[:, c0 * 2:(c0 + C_t) * 2].rearrange(
                    "p (c w) -> p c w", w=2),
                op=ALU.subtract)
            nc.vector.tensor_tensor(
                out=l_[:, :C_t * 2], in0=l_[:, :C_t * 2],
                in1=tev_t[:, c0 * 2:(c0 + C_t) * 2], op=ALU.add)
            r_ = wk.tile([128, MAXC * 2], dt.float32, tag="r")
            nc.scalar.activation(r_[:, :C_t * 2], l_[:, :C_t * 2], AF.Relu)
            rw = wk.tile([128, MAXC * 2], dt.float32, tag="rw")
            nc.vector.tensor_tensor(
                out=rw[:, :C_t * 2].rearrange("p (c w) -> p c w", w=2),
                in0=r_[:, :C_t * 2].rearrange("p (c w) -> p c w", w=2),
                in1=wd_t[:, 0:2].rearrange("p (o w) -> p o w", o=1).broadcast_to(
                    (128, C_t, 2)),
                op=ALU.mult)
            rw3 = rw[:, :C_t * 2].rearrange("p (c w) -> p c w", w=2)
            dd = wk.tile([128, MAXC], dt.float32, tag="dd")
            nc.vector.tensor_tensor(
                out=dd[:, :C_t].rearrange("p (c o) -> p c o", o=1),
                in0=rw3[:, :, 0:1], in1=rw3[:, :, 1:2], op=ALU.add)
            # delta = dd + bd; out0 = -softplus(-delta), out1 = -softplus(delta)
            # softplus(x) = relu(x) + ln(1 + exp(-|x|))
            dlt = wk.tile([128, MAXC], dt.float32, tag="dlt")
            nc.vector.tensor_scalar(out=dlt[:, :C_t], in0=dd[:, :C_t],
                                    scalar1=bd, scalar2=None, op0=ALU.add)
            ndl = wk.tile([128, MAXC], dt.float32, tag="ndl")
            nc.vector.tensor_scalar(out=ndl[:, :C_t], in0=dlt[:, :C_t],
                                    scalar1=-1.0, scalar2=None, op0=ALU.mult)
            ab = wk.tile([128, MAXC], dt.float32, tag="ab")
            nc.vector.tensor_tensor(out=ab[:, :C_t], in0=dlt[:, :C_t],
                                    in1=ndl[:, :C_t], op=ALU.max)
            en = wk.tile([128, MAXC], dt.float32, tag="en")
            nc.scalar.activation(en[:, :C_t], ab[:, :C_t], AF.Exp,
                                 scale=wd_t[:, 5:6])
            ep = wk.tile([128, MAXC], dt.float32, tag="ep")
            nc.vector.tensor_scalar(out=ep[:, :C_t], in0=en[:, :C_t],
                                    scalar1=1.0, scalar2=None, op0=ALU.add)
            lnp = wk.tile([128, MAXC], dt.float32, tag="lnp")
            nc.scalar.activation(lnp[:, :C_t], ep[:, :C_t], AF.Ln)
            r0 = wk.tile([128, MAXC], dt.float32, tag="r0")
            nc.scalar.activation(r0[:, :C_t], ndl[:, :C_t], AF.Relu)
            r1 = wk.tile([128, MAXC], dt.float32, tag="r1")
            nc.scalar.activation(r1[:, :C_t], dlt[:, :C_t], AF.Relu)
            s0 = wk.tile([128, MAXC], dt.float32, tag="s0")
            nc.vector.tensor_tensor(out=s0[:, :C_t], in0=r0[:, :C_t],
                                    in1=lnp[:, :C_t], op=ALU.add)
            s1 = wk.tile([128, MAXC], dt.float32, tag="s1")
            nc.vector.tensor_tensor(out=s1[:, :C_t], in0=r1[:, :C_t],
                                    in1=lnp[:, :C_t], op=ALU.add)
            ot = wk.tile([128, MAXC * 2], dt.float32, tag="ot")
            ot3 = ot[:, :C_t * 2].rearrange("p (c w) -> p c w", w=2)
            nc.vector.tensor_scalar(
                out=ot3[:, :, 0:1],
                in0=s0[:, :C_t].rearrange("p (c o) -> p c o", o=1),
                scalar1=-1.0, scalar2=None, op0=ALU.mult)
            nc.vector.tensor_scalar(
                out=ot3[:, :, 1:2],
                in0=s1[:, :C_t].rearrange("p (c o) -> p c o", o=1),
                scalar1=-1.0, scalar2=None, op0=ALU.mult)
            nc.sync.dma_start(t_out[:, c0 * 2:(c0 + C_t) * 2], ot[:, :C_t * 2])

    nc.compile()
    return nc


def kernel(x, edge_index, edge_attr, W1, b1, W2, b2, emb0, emb1,
           fcW1, fcb1, fcW2, fcb2, _prep_only=False):
    x = np.asarray(x, dtype=np.float32)
    W1 = np.asarray(W1, dtype=np.float32)
    b1 = np.asarray(b1, dtype=np.float32)
    W2 = np.asarray(W2, dtype=np.float32)
    fcW1 = np.asarray(fcW1, dtype=np.float32)
    fcb1 = np.asarray(fcb1, dtype=np.float32)
    fcW2 = np.asarray(fcW2, dtype=np.float32)
    fcb2 = np.asarray(fcb2, dtype=np.float32)
    emb0 = np.asarray(emb0, dtype=np.float32)
    emb1 = np.asarray(emb1, dtype=np.float32)

    percore, slotmaps, meta = _host_prep(
        x, np.asarray(edge_index), np.asarray(edge_attr),
        fcW1, fcb1, emb0, emb1)

    W2n = (W2 @ fcW1[:64]).astype(np.float32)
    iota128 = np.broadcast_to(np.arange(128, dtype=np.float32),
                              (128, 128)).astype(bf16).copy()
    ident128 = np.eye(128, dtype=np.float32).astype(bf16)
    wd = np.zeros((128, 6), dtype=np.float32)
    wd[:, 0] = fcW2[0, 0] - fcW2[0, 1]
    wd[:, 1] = fcW2[1, 0] - fcW2[1, 1]
    wd[:, 2] = fcb2[0] - fcb2[1]
    wd[:, 3] = -(fcb2[0] - fcb2[1])
    wd[:, 4] = 1.0
    wd[:, 5] = -1.0
    for m in percore:
        m["W1b"] = W1.astype(bf16)
        m["W2nb"] = W2n.astype(bf16)
        m["b1c"] = b1.reshape(F, 1).astype(np.float32)
        m["iota128"] = iota128
        m["ident128"] = ident128
        m["wd"] = wd
        m["idf"] = np.eye(2, dtype=np.float32)

    nc = _build(meta, fcW2, fcb2)
    if _prep_only:
        return nc, percore, slotmaps, meta

    res = run_bass_kernel_spmd(nc, percore, core_ids=list(range(NCORES)))
    outs = [np.asarray(res.results[k]["out"]) for k in range(NCORES)]
    return assemble(outs, slotmaps, meta)


def assemble(outs, slotmaps, meta):
    E = meta["E"]
    CSUM = meta["CSUM"]
    result = np.zeros((E, 2), dtype=np.float32)
    for k in range(NCORES):
        o = outs[k].reshape(128, CSUM, 2)
        sm = slotmaps[k]
        mask = sm >= 0
        result[sm[mask]] = o[mask]
    return result


# revision 15
# speedup vs baseline: 2.5646x; 1.1664x over previous
"""GCN-with-edge-MLP kernel for trn2, 8-core SPMD (self-contained).

Equal node shards (12544 nodes/core, row = node id everywhere). Per core:
  conv1: per 128-node dst tile, slots (edges+self, grouped by src quarter
         window so indices fit int16) gathered from x fp32 rows (256B) via
         4-queue dma_gather, scaled by host norm (dinv_s*dinv_d) on the Pool
         engine, aggregated with one-hot (is_equal) matmuls on PE ->
         h1 = relu(agg@W1 + b1), q = h1 @ (W2@fcW1[:64]).
  AllGather(q); expand q into 256B-strided quarter tables (strided DMA).
  conv2: same slot structure and the SAME int16 indices, gather q rows,
         2-wide aggregation -> p. Fused pd expansion: PE-transpose each
         one-hot chunk, pd_c = ohT_c^T @ p_tile, kept in SBUF.
  AllGather(p); expand p likewise.
  MLP in slot order: gather p rows (same indices) -> ps; l = ps - pd + TeV
         (TeV = host-folded edge-attr term, like the baseline's TeS fold);
         2-class log_softmax via softplus: out = (-sp(-d), -sp(d)).
  Host drops self/pad slots and permutes the slot-ordered output back to
  original edge order (index bookkeeping only).

Algebraic collapses (validated against the reference numerically):
  h1 = relu(agg(norm*x) @ W1 + b1),  q = h1 @ (W2 @ fcW1[:64])  (b2 cancels)
  logits1 = p[s]-p[d] + TeV[e]; out = log_softmax(relu(logits1)@fcW2 + fcb2)
"""
import numpy as np
import ml_dtypes

import concourse.bacc as bacc
import concourse.bass as bass
import concourse.mybir as mybir
import concourse.tile as tile
from concourse.bass_utils import run_bass_kernel_spmd
from contextlib import ExitStack

dt = mybir.dt
bf16 = ml_dtypes.bfloat16
NCORES = 8
AF = mybir.ActivationFunctionType
ALU = mybir.AluOpType

N_NODES = 100000
XP = 100352          # padded nodes = 8 * 12544 = 4 * 25088
NSH = XP // NCORES   # 12544 nodes per core
QW = XP // 4         # 25088-row quarter windows (int16-safe)
TPC = NSH // 128     # 98 tiles per core
F = 64
MAXC = 32            # SBUF capacity in 128-slot chunks per tile


def _host_prep(x, edge_index, edge_attr, fcW1, fcb1, emb0, emb1):
    N = x.shape[0]
    src = np.asarray(edge_index[0], dtype=np.int64)
    dst = np.asarray(edge_index[1], dtype=np.int64)
    ea = np.asarray(edge_attr, dtype=np.int64)
    E = src.shape[0]

    deg = np.bincount(dst, minlength=XP).astype(np.float32) + 1.0
    dinv = 1.0 / np.sqrt(deg)

    fcW1 = np.asarray(fcW1, dtype=np.float32)
    Te0 = (np.asarray(emb0, dtype=np.float32) @ fcW1[66:98])
    Te1 = (np.asarray(emb1, dtype=np.float32) @ fcW1[98:130])
    TeV = (Te0[ea[2]] + Te1[ea[3]]
           + ea[0][:, None].astype(np.float32) * fcW1[64][None, :]
           + ea[1][:, None].astype(np.float32) * fcW1[65][None, :]
           + np.asarray(fcb1, dtype=np.float32)[None, :]).astype(np.float32)

    order = np.argsort(dst, kind="stable")
    s_sorted, d_sorted = src[order], dst[order]
    e_sorted = order
    ptr = np.searchsorted(d_sorted, np.arange(XP + 1))

    xpad = np.zeros((XP, F), dtype=np.float32)
    xpad[:N] = np.asarray(x, dtype=np.float32)

    # pass 1: per (core, tile, quarter) slot lists and counts
    lists = [[[None] * 4 for _ in range(TPC)] for _ in range(NCORES)]
    CQ = np.zeros((TPC, 4), dtype=np.int64)   # common chunk capacity
    for k in range(NCORES):
        lo = k * NSH
        for t in range(TPC):
            n0 = lo + t * 128
            n1 = n0 + 128
            a, b = ptr[n0], ptr[n1]
            ss = np.concatenate([s_sorted[a:b], np.arange(n0, n1)])
            dd = np.concatenate([d_sorted[a:b], np.arange(n0, n1)])
            ee = np.concatenate([e_sorted[a:b], np.full(128, -1, np.int64)])
            qt = ss // QW
            for q in range(4):
                m = qt == q
                lists[k][t][q] = (ss[m], dd[m], ee[m])
                CQ[t, q] = max(CQ[t, q], (int(m.sum()) + 127) // 128)
    assert CQ.sum(axis=1).max() <= 30

    # common structure
    instr_meta = []   # (t, q, Cq, off16)
    off16 = 0
    tile_C = CQ.sum(axis=1).astype(np.int64)
    for t in range(TPC):
        for q in range(4):
            if CQ[t, q] == 0:
                continue
            instr_meta.append((t, q, int(CQ[t, q]), off16))
            off16 += int(CQ[t, q]) * 8   # (Cq*128)/16 int16 cols
    NIDX16 = off16
    CSUM = int(tile_C.sum())
    coff = np.concatenate([[0], np.cumsum(tile_C)]).astype(np.int64)

    # pass 2: per-core arrays at common capacities
    percore = []
    slotmaps = []
    for k in range(NCORES):
        idx_all = np.zeros((128, NIDX16), dtype=np.int16)
        dstloc = np.full((128, CSUM), 300.0, dtype=np.float32)
        normv = np.zeros((128, CSUM), dtype=np.float32)
        tev = np.zeros((128, CSUM, 2), dtype=np.float32)
        smap = np.full((128, CSUM), -1, dtype=np.int64)
        for (t, q, Cq, o16) in instr_meta:
            ss, dd, ee = lists[k][t][q]
            cnt = len(ss)
            cap = Cq * 128
            n0 = k * NSH + t * 128
            idx = np.zeros(cap, dtype=np.int16)
            idx[:cnt] = (ss - q * QW).astype(np.int16)
            w = idx.reshape(cap // 16, 16).T
            for g in range(8):
                idx_all[g * 16:(g + 1) * 16, o16:o16 + cap // 16] = w
            # column base for this quarter inside the tile
            cq0 = int(coff[t]) + int(CQ[t, :q].sum())
            kk = np.arange(cnt)
            pp, cc = kk % 128, cq0 + kk // 128
            dstloc[pp, cc] = (dd - n0).astype(np.float32)
            normv[pp, cc] = dinv[ss] * dinv[dd]
            real = ee >= 0
            tev[pp[real], cc[real]] = TeV[ee[real]]
            smap[pp, cc] = ee
        percore.append(dict(
            x=xpad, idx16=idx_all,
            dstloc=np.ascontiguousarray(dstloc.astype(bf16)),
            normv=np.ascontiguousarray(normv),
            tev=np.ascontiguousarray(tev.reshape(128, CSUM * 2))))
        slotmaps.append(smap)

    meta = dict(instr_meta=instr_meta, tile_C=tile_C, coff=coff,
                CSUM=CSUM, NIDX16=NIDX16, E=E)
    return percore, slotmaps, meta


def _build(meta, fcW2, fcb2):
    instr_meta = meta["instr_meta"]
    tile_C = meta["tile_C"]
    coff = meta["coff"]
    CSUM = meta["CSUM"]
    NIDX16 = meta["NIDX16"]

    wd0 = float(fcW2[0, 0] - fcW2[0, 1])
    wd1 = float(fcW2[1, 0] - fcW2[1, 1])
    bd = float(fcb2[0] - fcb2[1])

    nc = bacc.Bacc("TRN2", target_bir_lowering=False, debug=False,
                   num_devices=NCORES, num_swdge_queues=4)

    t_x = nc.dram_tensor("x", [XP, F], dt.float32, kind="ExternalInput")
    t_idx = nc.dram_tensor("idx16", [128, NIDX16], dt.int16, kind="ExternalInput")
    t_dl = nc.dram_tensor("dstloc", [128, CSUM], dt.bfloat16, kind="ExternalInput")
    t_nv = nc.dram_tensor("normv", [128, CSUM], dt.float32, kind="ExternalInput")
    t_tev = nc.dram_tensor("tev", [128, CSUM * 2], dt.float32, kind="ExternalInput")
    t_W1b = nc.dram_tensor("W1b", [F, F], dt.bfloat16, kind="ExternalInput")
    t_W2nb = nc.dram_tensor("W2nb", [F, 2], dt.bfloat16, kind="ExternalInput")
    t_b1c = nc.dram_tensor("b1c", [F, 1], dt.float32, kind="ExternalInput")
    t_iota = nc.dram_tensor("iota128", [128, 128], dt.bfloat16, kind="ExternalInput")
    t_ident = nc.dram_tensor("ident128", [128, 128], dt.bfloat16, kind="ExternalInput")
    t_wd = nc.dram_tensor("wd", [128, 6], dt.float32, kind="ExternalInput")
    t_idf = nc.dram_tensor("idf", [2, 2], dt.float32, kind="ExternalInput")

    t_out = nc.dram_tensor("out", [128, CSUM * 2], dt.float32, kind="ExternalOutput")


    d_qloc = nc.dram_tensor("qloc_i", [NSH, 2], dt.float32)
    d_qfull = nc.dram_tensor("qfull_i", [XP, 2], dt.float32, addr_space="Shared")
    d_ploc = nc.dram_tensor("ploc_i", [NSH, 2], dt.float32)
    d_pfull = nc.dram_tensor("pfull_i", [XP, 2], dt.float32, addr_space="Shared")
    d_qexp = [nc.dram_tensor(f"qexp{i}", [QW, F], dt.float32) for i in range(4)]
    d_pexp = [nc.dram_tensor(f"pexp{i}", [QW, F], dt.float32) for i in range(4)]

    tile_instrs = [[] for _ in range(TPC)]
    for (t, q, Cq, o16) in instr_meta:
        tile_instrs[t].append((q, Cq, o16))

    with tile.TileContext(nc) as tc, ExitStack() as ctx:
        cst = ctx.enter_context(tc.tile_pool(name="cst", bufs=1))
        wk = ctx.enter_context(tc.tile_pool(name="wk", bufs=2))
        gp = ctx.enter_context(tc.tile_pool(name="gp", bufs=2))
        ohp = ctx.enter_context(tc.tile_pool(name="ohp", bufs=2))
        psA = ctx.enter_context(tc.tile_pool(name="psA", bufs=3, space="PSUM"))
        psB = ctx.enter_context(tc.tile_pool(name="psB", bufs=1, space="PSUM"))
        psC = ctx.enter_context(tc.tile_pool(name="psC", bufs=2, space="PSUM"))
        psD = ctx.enter_context(tc.tile_pool(name="psD", bufs=2, space="PSUM"))

        iota_t = cst.tile([128, 128], dt.bfloat16, tag="iota")
        nc.sync.dma_start(iota_t[:], t_iota[:, :])
        ident_t = cst.tile([128, 128], dt.bfloat16, tag="ident")
        nc.sync.dma_start(ident_t[:], t_ident[:, :])
        idf_t = cst.tile([2, 2], dt.float32, tag="idf")
        nc.sync.dma_start(idf_t[:], t_idf[:, :])
        W1b_t = cst.tile([F, F], dt.bfloat16, tag="W1b")
        nc.sync.dma_start(W1b_t[:], t_W1b[:, :])
        W2nb_t = cst.tile([F, 2], dt.bfloat16, tag="W2nb")
        nc.sync.dma_start(W2nb_t[:], t_W2nb[:, :])
        b1c_t = cst.tile([F, 1], dt.float32, tag="b1c")
        nc.sync.dma_start(b1c_t[:], t_b1c[:, :])
        wd_t = cst.tile([128, 6], dt.float32, tag="wd")
        nc.sync.dma_start(wd_t[:], t_wd[:, :])

        idx_t = cst.tile([128, NIDX16], dt.int16, tag="idx")
        nc.sync.dma_start(idx_t[:], t_idx[:, :])
        dl_t = cst.tile([128, CSUM], dt.bfloat16, tag="dl")
        nc.sync.dma_start(dl_t[:], t_dl[:, :])
        nv_t = cst.tile([128, CSUM], dt.float32, tag="nv")
        nc.sync.dma_start(nv_t[:], t_nv[:, :])
        tev_t = cst.tile([128, CSUM * 2], dt.float32, tag="tev")
        nc.sync.dma_start(tev_t[:], t_tev[:, :])
        pdv_t = cst.tile([128, CSUM * 2], dt.float32, tag="pdv")

        def gathers(t, table_list):
            g = gp.tile([128, MAXC * F], dt.float32, tag="g")
            co = 0
            for gi, (q, Cq, o16) in enumerate(tile_instrs[t]):
                n = Cq * 128
                tab = table_list[q]
                nc.gpsimd.dma_gather(
                    out_ap=g[:, co * F:(co + Cq) * F].rearrange(
                        "p (c f) -> p c f", f=F),
                    in_ap=tab if isinstance(tab, bass.AP) else tab[:, :],
                    idxs_ap=idx_t[:, o16:o16 + n // 16],
                    num_idxs=n, num_idxs_reg=n, elem_size=F,
                    queue_num=(t * 4 + gi) % 4)
                co += Cq
            return g

        def build_oh(t, C_t, c0):
            oh = ohp.tile([128, MAXC * 128], dt.bfloat16, tag="oh")
            nc.vector.tensor_tensor(
                out=oh[:, :C_t * 128].rearrange("p (c j) -> p c j", j=128),
                in0=dl_t[:, c0:c0 + C_t].broadcast_to((128, C_t, 128)),
                in1=iota_t[:].rearrange("p (o j) -> p o j", o=1).broadcast_to(
                    (128, C_t, 128)),
                op=ALU.is_equal)
            return oh

        # ---------------- conv1 ----------------
        for t in range(TPC):
            C_t = int(tile_C[t])
            c0 = int(coff[t])
            g = gathers(t, [t_x[i * QW:(i + 1) * QW, :] for i in range(4)])
            gbf = wk.tile([128, MAXC * F], dt.bfloat16, tag="gbf")
            nc.vector.tensor_tensor(
                out=gbf[:, :C_t * F].rearrange("p (c f) -> p c f", f=F),
                in0=g[:, :C_t * F].rearrange("p (c f) -> p c f", f=F),
                in1=nv_t[:, c0:c0 + C_t].rearrange(
                    "p (c o) -> p c o", o=1).broadcast_to((128, C_t, F)),
                op=ALU.mult)
            oh = build_oh(t, C_t, c0)
            aggT = psA.tile([F, 128], dt.float32, tag="p64")
            for c in range(C_t):
                nc.tensor.matmul(
                    out=aggT[:], lhsT=gbf[:, c * F:(c + 1) * F],
                    rhs=oh[:, c * 128:(c + 1) * 128],
                    start=(c == 0), stop=(c == C_t - 1))
            aggS = wk.tile([F, 128], dt.bfloat16, tag="aggS")
            nc.vector.tensor_copy(aggS[:], aggT[:])
            h1p = psA.tile([F, 128], dt.float32, tag="p64")
            nc.tensor.matmul(out=h1p[:], lhsT=W1b_t[:], rhs=aggS[:],
                             start=True, stop=True)
            h1r = wk.tile([F, 128], dt.bfloat16, tag="h1r")
            nc.scalar.activation(h1r[:], h1p[:], AF.Relu, bias=b1c_t[:, 0:1])
            qTp = psB.tile([2, 128], dt.float32, tag="p2")
            nc.tensor.matmul(out=qTp[:], lhsT=W2nb_t[:], rhs=h1r[:],
                             start=True, stop=True)
            qTs = wk.tile([2, 128], dt.float32, tag="qTs")
            nc.vector.tensor_copy(qTs[:], qTp[:])
            qmix = psC.tile([128, 64], dt.float32, tag="pMix")
            nc.tensor.transpose(out=qmix[:, 62:64], in_=qTs[:],
                                identity=idf_t[0:2, 0:2])
            qsb = wk.tile([128, 2], dt.float32, tag="qsb")
            nc.vector.tensor_copy(qsb[:], qmix[:, 62:64])
            nc.sync.dma_start(d_qloc[t * 128:(t + 1) * 128, 0:2], qsb[:])
        tc.strict_bb_all_engine_barrier()

        nc.gpsimd.collective_compute(
            "AllGather", ALU.bypass, replica_groups=[list(range(NCORES))],
            ins=[d_qloc[:, :].opt()], outs=[d_qfull[:, :].opt()])
        tc.strict_bb_all_engine_barrier()
        _eng = [nc.sync, nc.scalar, nc.sync, nc.scalar]
        for i in range(4):
            _eng[i].dma_start(d_qexp[i][:, 0:2], d_qfull[i * QW:(i + 1) * QW, :])
        tc.strict_bb_all_engine_barrier()

        # ---------------- conv2 + fused pd ----------------
        for t in range(TPC):
            C_t = int(tile_C[t])
            c0 = int(coff[t])
            g = gathers(t, d_qexp)
            g2n = wk.tile([128, MAXC * 2], dt.bfloat16, tag="g2n")
            nc.vector.tensor_tensor(
                out=g2n[:, :C_t * 2].rearrange("p (c w) -> p c w", w=2),
                in0=g[:, :C_t * F].rearrange("p (c f) -> p c f", f=F)[:, :, 0:2],
                in1=nv_t[:, c0:c0 + C_t].rearrange(
                    "p (c o) -> p c o", o=1).broadcast_to((128, C_t, 2)),
                op=ALU.mult)
            oh = build_oh(t, C_t, c0)
            aggT2 = psB.tile([2, 128], dt.float32, tag="p2")
            for c in range(C_t):
                nc.tensor.matmul(
                    out=aggT2[:], lhsT=g2n[:, c * 2:(c + 1) * 2],
                    rhs=oh[:, c * 128:(c + 1) * 128],
                    start=(c == 0), stop=(c == C_t - 1))
            a2s = wk.tile([2, 128], dt.float32, tag="a2s")
            nc.vector.tensor_copy(a2s[:], aggT2[:])
            pdp = psC.tile([128, 64], dt.float32, tag="pMix")
            nc.tensor.transpose(out=pdp[:, 62:64], in_=a2s[:],
                                identity=idf_t[0:2, 0:2])
            p_sb = wk.tile([128, 2], dt.float32, tag="psb")
            nc.vector.tensor_copy(p_sb[:], pdp[:, 62:64])
            nc.sync.dma_start(d_ploc[t * 128:(t + 1) * 128, 0:2], p_sb[:])
            p_bf = wk.tile([128, 2], dt.bfloat16, tag="pbf")
            nc.vector.tensor_copy(p_bf[:], pdp[:, 62:64])
            for c in range(C_t):
                ohTp = psD.tile([128, 128], dt.bfloat16, tag="ohTp")
                nc.tensor.transpose(out=ohTp[:], in_=oh[:, c * 128:(c + 1) * 128],
                                    identity=ident_t[:])
                ohTs = ohp.tile([128, 128], dt.bfloat16, tag="ohTs")
                nc.vector.tensor_copy(ohTs[:], ohTp[:])
                nc.tensor.matmul(out=pdp[:, c * 2:(c + 1) * 2], lhsT=ohTs[:],
                                 rhs=p_bf[:], start=True, stop=True)
            nc.vector.tensor_copy(pdv_t[:, c0 * 2:(c0 + C_t) * 2],
                                  pdp[:, :C_t * 2])
        tc.strict_bb_all_engine_barrier()

        nc.gpsimd.collective_compute(
            "AllGather", ALU.bypass, replica_groups=[list(range(NCORES))],
            ins=[d_ploc[:, :].opt()], outs=[d_pfull[:, :].opt()])
        tc.strict_bb_all_engine_barrier()
        for i in range(4):
            _eng[i].dma_start(d_pexp[i][:, 0:2], d_pfull[i * QW:(i + 1) * QW, :])
        tc.strict_bb_all_engine_barrier()

        # ---------------- MLP (slot order) ----------------
        for t in range(TPC):
            C_t = int(tile_C[t])
            c0 = int(coff[t])
            g = gathers(t, d_pexp)
            ps3 = g[:, :C_t * F].rearrange("p (c f) -> p c f", f=F)[:, :, 0:2]
            l_ = wk.tile([128, MAXC * 2], dt.float32, tag="l")
            l3 = l_[:, :C_t * 2].rearrange("p (c w) -> p c w", w=2)
            nc.vector.tensor_tensor(
                out=l3, in0=ps3,
                in1=pdv_t[:, c0 * 2:(c0 + C_t) * 2].rearrange(
                    "p (c w) -> p c w", w=2),
                op=ALU.subtract)
            nc.vector.tensor_tensor(
                out=l_[:, :C_t * 2], in0=l_[:, :C_t * 2],
                in1=tev_t[:, c0 * 2:(c0 + C_t) * 2], op=ALU.add)
            r_ = wk.tile([128, MAXC * 2], dt.float32, tag="r")
            nc.scalar.activation(r_[:, :C_t * 2], l_[:, :C_t * 2], AF.Relu)
            rw = wk.tile([128, MAXC * 2], dt.float32, tag="rw")
            nc.vector.tensor_tensor(
                out=rw[:, :C_t * 2].rearrange("p (c w) -> p c w", w=2),
                in0=r_[:, :C_t * 2].rearrange("p (c w) -> p c w", w=2),
                in1=wd_t[:, 0:2].rearrange("p (o w) -> p o w", o=1).broadcast_to(
                    (128, C_t, 2)),
                op=ALU.mult)
            rw3 = rw[:, :C_t * 2].rearrange("p (c w) -> p c w", w=2)
            dd = wk.tile([128, MAXC], dt.float32, tag="dd")
            nc.vector.tensor_tensor(
                out=dd[:, :C_t].rearrange("p (c o) -> p c o", o=1),
                in0=rw3[:, :, 0:1], in1=rw3[:, :, 1:2], op=ALU.add)
            # delta = dd + bd; out0 = -softplus(-delta), out1 = -softplus(delta)
            # softplus(x) = relu(x) + ln(1 + exp(-|x|))
            dlt = wk.tile([128, MAXC], dt.float32, tag="dlt")
            nc.vector.tensor_scalar(out=dlt[:, :C_t], in0=dd[:, :C_t],
                                    scalar1=bd, scalar2=None, op0=ALU.add)
            ndl = wk.tile([128, MAXC], dt.float32, tag="ndl")
            nc.vector.tensor_scalar(out=ndl[:, :C_t], in0=dlt[:, :C_t],
                                    scalar1=-1.0, scalar2=None, op0=ALU.mult)
            ab = wk.tile([128, MAXC], dt.float32, tag="ab")
            nc.vector.tensor_tensor(out=ab[:, :C_t], in0=dlt[:, :C_t],
                                    in1=ndl[:, :C_t], op=ALU.max)
            en = wk.tile([128, MAXC], dt.float32, tag="en")
            nc.scalar.activation(en[:, :C_t], ab[:, :C_t], AF.Exp,
                                 scale=wd_t[:, 5:6])
            ep = wk.tile([128, MAXC], dt.float32, tag="ep")
            nc.vector.tensor_scalar(out=ep[:, :C_t], in0=en[:, :C_t],
                                    scalar1=1.0, scalar2=None, op0=ALU.add)
            lnp = wk.tile([128, MAXC], dt.float32, tag="lnp")
            nc.scalar.activation(lnp[:, :C_t], ep[:, :C_t], AF.Ln)
            r0 = wk.tile([128, MAXC], dt.float32, tag="r0")
            nc.scalar.activation(r0[:, :C_t], ndl[:, :C_t], AF.Relu)
            r1 = wk.tile([128, MAXC], dt.float32, tag="r1")
            nc.scalar.activation(r1[:, :C_t], dlt[:, :C_t], AF.Relu)
            s0 = wk.tile([128, MAXC], dt.float32, tag="s0")
            nc.vector.tensor_tensor(out=s0[:, :C_t], in0=r0[:, :C_t],
                                    in1=lnp[:, :C_t], op=ALU.add)
            s1 = wk.tile([128, MAXC], dt.float32, tag="s1")
            nc.vector.tensor_tensor(out=s1[:, :C_t], in0=r1[:, :C_t],
                                    in1=lnp[:, :C_t], op=ALU.add)
            ot = wk.tile([128, MAXC * 2], dt.float32, tag="ot")
            ot3 = ot[:, :C_t * 2].rearrange("p (c w) -> p c w", w=2)
            nc.vector.tensor_scalar(
                out=ot3[:, :, 0:1],
                in0=s0[:, :C_t].rearrange("p (c o) -> p c o", o=1),
                scalar1=-1.0, scalar2=None, op0=ALU.mult)
            nc.vector.tensor_scalar(
                out=ot3[:, :, 1:2],
                in0=s1[:, :C_t].rearrange("p (c o) -> p c o", o=1),
                scalar1=-1.0, scalar2=None, op0=ALU.mult)
            nc.sync.dma_start(t_out[:, c0 * 2:(c0 + C_t) * 2], ot[:, :C_t * 2])

    nc.compile()
    return nc


def kernel(x, edge_index, edge_attr, W1, b1, W2, b2, emb0, emb1,
           fcW1, fcb1, fcW2, fcb2, _prep_only=False):
    x = np.asarray(x, dtype=np.float32)
    W1 = np.asarray(W1, dtype=np.float32)
    b1 = np.asarray(b1, dtype=np.float32)
    W2 = np.asarray(W2, dtype=np.float32)
    fcW1 = np.asarray(fcW1, dtype=np.float32)
    fcb1 = np.asarray(fcb1, dtype=np.float32)
    fcW2 = np.asarray(fcW2, dtype=np.float32)
    fcb2 = np.asarray(fcb2, dtype=np.float32)
    emb0 = np.asarray(emb0, dtype=np.float32)
    emb1 = np.asarray(emb1, dtype=np.float32)

    percore, slotmaps, meta = _host_prep(
        x, np.asarray(edge_index), np.asarray(edge_attr),
        fcW1, fcb1, emb0, emb1)

    W2n = (W2 @ fcW1[:64]).astype(np.float32)
    iota128 = np.broadcast_to(np.arange(128, dtype=np.float32),
                              (128, 128)).astype(bf16).copy()
    ident128 = np.eye(128, dtype=np.float32).astype(bf16)
    wd = np.zeros((128, 6), dtype=np.float32)
    wd[:, 0] = fcW2[0, 0] - fcW2[0, 1]
    wd[:, 1] = fcW2[1, 0] - fcW2[1, 1]
    wd[:, 2] = fcb2[0] - fcb2[1]
    wd[:, 3] = -(fcb2[0] - fcb2[1])
    wd[:, 4] = 1.0
    wd[:, 5] = -1.0
    for m in percore:
        m["W1b"] = W1.astype(bf16)
        m["W2nb"] = W2n.astype(bf16)
        m["b1c"] = b1.reshape(F, 1).astype(np.float32)
        m["iota128"] = iota128
        m["ident128"] = ident128
        m["wd"] = wd
        m["idf"] = np.eye(2, dtype=np.float32)

    nc = _build(meta, fcW2, fcb2)
    if _prep_only:
        return nc, percore, slotmaps, meta

    res = run_bass_kernel_spmd(nc, percore, core_ids=list(range(NCORES)))
    outs = [np.asarray(res.results[k]["out"]) for k in range(NCORES)]
    return assemble(outs, slotmaps, meta)


def assemble(outs, slotmaps, meta):
    E = meta["E"]
    CSUM = meta["CSUM"]
    result = np.zeros((E, 2), dtype=np.float32)
    for k in range(NCORES):
        o = outs[k].reshape(128, CSUM, 2)
        sm = slotmaps[k]
        mask = sm >= 0
        result[sm[mask]] = o[mask]
    return result
